# revision 5
# baseline (speedup 1.0000x reference)
"""Trainium2 Bass kernel for nn_BertEmbedding_1623497638029.

Per batch row b and token t (T=256 tokens, P=512 subword positions,
H=768), with subword counts lens in {0,1,2}:

    cum  = cumsum(bert_lens[b])
    lo_t = cum[t] - lens[t]        # first subword of token t
    hi_t = cum[t] - 1              # last subword of token t
    out[b,t] = mean(enc[b, lo_t:hi_t+1]) if lens[t] else 0

Implementation: tokens are processed as 128 PAIRS per batch row. One
indirect-DMA gather per batch row fetches a 4-row window per pair,
rows w..w+3 with w = max(cum[2p+1]-4, 0) (always in bounds since
cum <= P). The window covers both tokens' subword rows; each token's
output is a coefficient-weighted sum of the 4 window rows:

    m1 = min(cum[2p+1]-4, 0)            # window clamp shift (rarely nonzero)
    pos_hi1 = 3+m1           pos_lo1 = 4-l1+m1
    pos_hi0 = 3-l1+m1        pos_lo0 = 4-l1-l0+m1
    out_even = sum_k u_k r_k,  u_k = b0*[pos_hi0==k] + a0*[pos_lo0==k]
    out_odd  = sum_k v_k r_k,  v_k = b1*[pos_hi1==k] + a1*[pos_lo1==k]
    a = 0.5*(len>1),  b = (len>0) - a   (per token)

At most two coefficients per token are nonzero (0.5/0.5 for len==2,
one 1.0 for len==1, none for padding), so every output element is a
single-rounded sum of at most two scaled rows — bit-identical to the
reference segment-mean.

The pair-layout quantities cum[2p+1], lens[2p], lens[2p+1] are computed
on-chip with selection-matrix matmuls (PE) from a transposed lens tile;
the selection masks are affine_select constants.

This shape (8 gathers of 1.5 MB with 12 KB descriptors + 4 contiguous
1.5 MB stores per core, two batch rows per store) measured fastest on
hardware (59.8 us median vs 61-63 us with 8 single-row stores) versus
per-token
2-row gathers (16 DMAs), per-row gathers with OOB-skipped accumulation
(32 DMAs), and bounds-checked window skipping of padding pairs
(bounds_check + oob_is_err=False: +10 us -- the checked SWDGE path
costs more per descriptor than the ~11% byte saving recovers).
dma_gather (InstDMAGatherAnt) crashes at device execution in this
axon environment, and multi-index offset APs ([128, k>1]) are not
honored by the HW ucode (only offset[p,0] is used).

Pool depths acc_bufs=8 / res_bufs=4 / tmp_bufs=4 measured ~6% faster
than 6/4/2 (61.0 us vs 65.0 us median) and faster than 10/4/2
(62.6 us) -- both gather prefetch depth and combine tmp depth matter.
Timing-loop max_unroll=4 measured equal-best (61.2 us mid-5 vs 61.4/
62.5 at unroll 2). Component floors: gather-only 37.8 us for the
12.58 MB read (333 GB/s = 93% of the 358 GB/s per-core HBM limit),
store-only 20.9 us for 6.29 MB; the full kernel runs within ~2-4 us
of their serial sum -- reads and writes share HBM, so this is the
practical floor for this traffic volume.

Sharding: pure data parallel — 8 batch rows per NeuronCore, 8 cores,
no cross-core communication.
"""

import numpy as np

import concourse.bacc as bacc
import concourse.bass as bass
import concourse.mybir as mybir
import concourse.tile as tile
from concourse.bass_utils import run_bass_kernel_spmd
from concourse.masks import make_identity

NCORES = 8
BZ, P, T, H = 64, 512, 256, 768
BL = BZ // NCORES  # batch rows per core
NCH = T // 128  # 128-token chunks per batch row

F32 = mybir.dt.float32
BF16 = mybir.dt.bfloat16
I32 = mybir.dt.int32
ALU = mybir.AluOpType
AF = mybir.ActivationFunctionType


def _sel_mask(nc, t, base):
    """t[k,p] = 1 iff base + k - 2p == 0."""
    nc.gpsimd.memset(t, 0.0)
    nc.gpsimd.affine_select(
        out=t, in_=t, compare_op=ALU.not_equal, fill=1.0,
        base=base, pattern=[[-2, 128]], channel_multiplier=1,
    )


def _le_mask(nc, t, base):
    """t[k,p] = 1 iff base + k - 2p <= 0."""
    nc.gpsimd.memset(t, 0.0)
    nc.gpsimd.affine_select(
        out=t, in_=t, compare_op=ALU.is_gt, fill=1.0,
        base=base, pattern=[[-2, 128]], channel_multiplier=1,
    )


def _build_nc(acc_bufs=8, res_bufs=2, tmp_bufs=4, repeat=0, asserts=True,
              pool_add=False):
    nc = bacc.Bacc(
        "TRN2", target_bir_lowering=False, debug=False,
        num_devices=NCORES, enable_asserts=asserts,
    )
    enc = nc.dram_tensor("enc", [BL * P, H], F32, kind="ExternalInput").ap()
    lens = nc.dram_tensor("lens", [BL, T], I32, kind="ExternalInput").ap()
    out = nc.dram_tensor("out", [BL, T, H], BF16, kind="ExternalOutput").ap()

    with tile.TileContext(nc) as tc:
        with (
            tc.tile_pool(name="const", bufs=1) as cpool,
            tc.tile_pool(name="idx", bufs=1) as ipool,
            tc.tile_pool(name="psum", bufs=2, space="PSUM") as ppool,
            tc.tile_pool(name="acc", bufs=acc_bufs) as apool,
            tc.tile_pool(name="tmp", bufs=tmp_bufs) as tpool,
            tc.tile_pool(name="res", bufs=res_bufs) as rpool,
        ):
            # ---- constants ----
            ident = cpool.tile([128, 128], F32)
            make_identity(nc, ident[:])
            # selection/prefix masks: columns are pairs p, rows are chunk-local k
            mcum0 = cpool.tile([128, 128], F32)  # k <= 2p+1 (chunk 0)
            _le_mask(nc, mcum0[:], base=-1)
            mcum1 = cpool.tile([128, 128], F32)  # 128+k <= 2p+1
            _le_mask(nc, mcum1[:], base=127)
            se0 = cpool.tile([128, 128], F32)  # k == 2p (chunk 0)
            _sel_mask(nc, se0[:], base=0)
            se1 = cpool.tile([128, 128], F32)  # 128+k == 2p
            _sel_mask(nc, se1[:], base=128)
            so0 = cpool.tile([128, 128], F32)  # k == 2p+1 (chunk 0)
            _sel_mask(nc, so0[:], base=-1)
            so1 = cpool.tile([128, 128], F32)  # 128+k == 2p+1
            _sel_mask(nc, so1[:], base=127)
            boff_i = cpool.tile([128, BL], I32)  # per-column batch row offset
            nc.gpsimd.iota(boff_i[:], pattern=[[P, BL]], base=0, channel_multiplier=0)
            boff = cpool.tile([128, BL], F32)
            nc.vector.tensor_copy(boff[:], boff_i[:])
            badj = cpool.tile([1, BL], F32)  # 512*n - 4 per column
            nc.vector.tensor_scalar_add(badj[:], boff[0:1, :], -4.0)

            # ---- load lens, cast, transpose chunks to [token, batch] ----
            lens_i = ipool.tile([BL, T], I32)
            nc.sync.dma_start(out=lens_i[:], in_=lens[:, :])
            lens_f = ipool.tile([BL, T], F32)
            nc.vector.tensor_copy(lens_f[:], lens_i[:])
            lensT = []
            for c in range(NCH):
                ps_t = ppool.tile([128, BL], F32, tag="tr")
                nc.tensor.transpose(
                    out=ps_t[:], in_=lens_f[:, c * 128 : (c + 1) * 128],
                    identity=ident[0:BL, 0:BL],
                )
                lt = ipool.tile([128, BL], F32, tag=f"lensT{c}")
                nc.vector.tensor_copy(lt[:], ps_t[:])
                lensT.append(lt)

            # ---- pair-layout quantities via selection matmuls ----
            def _accum(masks, name, extra=None):
                pt = ppool.tile([128, BL], F32, tag=name)
                nc.tensor.matmul(out=pt[:], lhsT=masks[0][:], rhs=lensT[0][:],
                                 start=True, stop=False)
                nc.tensor.matmul(out=pt[:], lhsT=masks[1][:], rhs=lensT[1][:],
                                 start=False, stop=(extra is None))
                if extra is not None:
                    # rank-1 accumulate: mcum0 row 0 is all ones
                    nc.tensor.matmul(out=pt[:], lhsT=mcum0[0:1, :],
                                     rhs=extra[:], start=False, stop=True)
                return pt

            c1v = _accum((mcum0, mcum1), "c1v", extra=badj)  # cum[2p+1]-4+boff
            l0v = _accum((se0, se1), "l0v")      # lens[2p]
            l1v = _accum((so0, so1), "l1v")      # lens[2p+1]

            # ---- window index and coefficients (all [128, BL] f32) ----
            # c1v holds cum-4+boff; wg = max(cum-4,0)+boff = max(c1v, boff)
            wg = ipool.tile([128, BL], I32)
            nc.vector.tensor_tensor(out=wg[:], in0=c1v[:], in1=boff[:],
                                    op=ALU.max)
            m1 = ipool.tile([128, BL], F32)  # min(cum-4, 0) = min(c1v-boff, 0)
            nc.vector.tensor_sub(m1[:], c1v[:], boff[:])
            nc.vector.tensor_scalar_min(m1[:], m1[:], 0.0)

            x = ipool.tile([128, BL], F32)  # m1 - l1
            nc.vector.tensor_sub(x[:], m1[:], l1v[:])
            pos_hi1 = ipool.tile([128, BL], F32)
            nc.vector.tensor_scalar_add(pos_hi1[:], m1[:], 3.0)
            pos_hi0 = ipool.tile([128, BL], F32)
            nc.vector.tensor_scalar_add(pos_hi0[:], x[:], 3.0)
            pos_lo1 = ipool.tile([128, BL], F32)
            nc.vector.tensor_scalar_add(pos_lo1[:], x[:], 4.0)
            pos_lo0 = ipool.tile([128, BL], F32)
            nc.vector.tensor_sub(pos_lo0[:], pos_lo1[:], l0v[:])

            def ab(lv, tag):
                a = ipool.tile([128, BL], F32, tag=f"a{tag}")
                nc.vector.tensor_scalar(out=a[:], in0=lv[:], scalar1=1.0,
                                        scalar2=0.5, op0=ALU.is_gt, op1=ALU.mult)
                g = ipool.tile([128, BL], F32, tag=f"g{tag}")
                nc.vector.tensor_scalar(out=g[:], in0=lv[:], scalar1=0.0,
                                        scalar2=None, op0=ALU.is_gt)
                b = ipool.tile([128, BL], F32, tag=f"b{tag}")
                nc.vector.tensor_sub(b[:], g[:], a[:])
                return a, b

            a0, b0 = ab(l0v, "0")
            a1, b1 = ab(l1v, "1")

            def coef(k, poshi, poslo, av, bv, tag):
                ih = ipool.tile([128, BL], F32, tag=f"ih{tag}{k}")
                nc.vector.tensor_scalar(out=ih[:], in0=poshi[:],
                                        scalar1=float(k), scalar2=None,
                                        op0=ALU.is_equal)
                il = ipool.tile([128, BL], F32, tag=f"il{tag}{k}")
                nc.vector.tensor_scalar(out=il[:], in0=poslo[:],
                                        scalar1=float(k), scalar2=None,
                                        op0=ALU.is_equal)
                u = ipool.tile([128, BL], F32, tag=f"u{tag}{k}")
                nc.vector.tensor_mul(u[:], ih[:], bv[:])
                t2 = ipool.tile([128, BL], F32, tag=f"t2{tag}{k}")
                nc.vector.tensor_mul(t2[:], il[:], av[:])
                nc.vector.tensor_add(u[:], u[:], t2[:])
                return u

            u = [coef(k, pos_hi0, pos_lo0, a0, b0, "u") for k in range(4)]
            v = [coef(k, pos_hi1, pos_lo1, a1, b1, "v") for k in range(4)]

            # ---- main loop: one 4-row-window gather + combine per batch;
            # results for two batch rows share one res tile so each store
            # covers 1.5 MB (halves store-instruction count) ----
            def main_body(_iv=None):
                res = None
                for b in range(BL):
                    acc = apool.tile([128, 4 * H], F32, tag="acc")
                    nc.gpsimd.indirect_dma_start(
                        out=acc[:], out_offset=None, in_=enc[:, :],
                        in_offset=bass.IndirectOffsetOnAxis(
                            ap=wg[:, b : b + 1], axis=0),
                    )
                    if b % 2 == 0:
                        res = rpool.tile([128, 4 * H], BF16, tag="res")
                    half = (b % 2) * 2 * H
                    r = [acc[:, k * H : (k + 1) * H] for k in range(4)]
                    for parity, cf in ((0, u), (1, v)):
                        sl = res[:, half + parity * H : half + (parity + 1) * H]
                        # sl = cf0*r0 + (cf1*r1 + (cf2*r2 + cf3*r3))
                        t3 = tpool.tile([128, H], F32, tag=f"t3{parity}")
                        nc.scalar.activation(out=t3[:], in_=r[3], func=AF.Copy,
                                             scale=cf[3][:, b : b + 1])
                        t2 = tpool.tile([128, H], F32, tag=f"t2{parity}")
                        nc.vector.scalar_tensor_tensor(
                            out=t2[:], in0=r[2], scalar=cf[2][:, b : b + 1],
                            in1=t3[:], op0=ALU.mult, op1=ALU.add)
                        t1 = tpool.tile([128, H], F32, tag=f"t1{parity}")
                        if pool_add:
                            nc.scalar.activation(out=t1[:], in_=r[1],
                                                 func=AF.Copy,
                                                 scale=cf[1][:, b : b + 1])
                            nc.gpsimd.tensor_add(t1[:], t1[:], t2[:])
                        else:
                            nc.vector.scalar_tensor_tensor(
                                out=t1[:], in0=r[1],
                                scalar=cf[1][:, b : b + 1],
                                in1=t2[:], op0=ALU.mult, op1=ALU.add)
                        nc.vector.scalar_tensor_tensor(
                            out=sl, in0=r[0], scalar=cf[0][:, b : b + 1],
                            in1=t1[:], op0=ALU.mult, op1=ALU.add)
                    if b % 2 == 1:
                        dest = out[b - 1 : b + 1, :, :].rearrange(
                            "o (tp q) h -> tp o q h", q=2
                        )
                        nc.sync.dma_start(out=dest, in_=res[:].rearrange(
                            "p (o q h) -> p o q h", q=2, h=H))

            if repeat:
                # timing mode: run the steady-state body `repeat` times
                tc.For_i_unrolled(0, repeat, 1, main_body, max_unroll=4)
            else:
                main_body()

    nc.compile()
    return nc


_NC = None


def _get_nc():
    global _NC
    if _NC is None:
        _NC = _build_nc()
    return _NC


def kernel(enc_out, bert_mask, bert_lens):
    del bert_mask  # implied by bert_lens (mask = arange(P) < cumsum(lens)[-1])
    enc_np = np.ascontiguousarray(np.asarray(enc_out, dtype=np.float32))
    lens_np = np.ascontiguousarray(np.asarray(bert_lens, dtype=np.int32))
    assert enc_np.shape == (BZ, P, H) and lens_np.shape == (BZ, T)

    nc = _get_nc()
    in_maps = [
        {
            "enc": enc_np[i * BL : (i + 1) * BL].reshape(BL * P, H),
            "lens": lens_np[i * BL : (i + 1) * BL],
        }
        for i in range(NCORES)
    ]
    results = run_bass_kernel_spmd(nc, in_maps, core_ids=list(range(NCORES))).results
    out = np.concatenate(
        [np.asarray(r["out"]).astype(np.float32) for r in results], axis=0
    )
    return out.reshape(BZ, T, H)



# revision 7
# speedup vs baseline: 2.1095x; 2.1095x over previous
"""Trainium2 Bass kernel for nn_BertEmbedding_1623497638029 (PE version).

Per batch row b and token t (T=256 tokens, P=512 subword positions,
H=768), with subword counts lens in {0,1,2}:

    out[b,t] = mean(enc[b, cum[t]-lens[t] : cum[t]]) if lens[t] else 0

Strategy (pure data parallel, 8 rows/core, no cross-core comms):

  * Loads: full-128-offset indirect DMA (iota offsets, sequential rows).
    Measured DMA rates on this hardware for the same bytes/descriptors:
    indirect+128 offsets 353 GB/s; indirect with <128 offsets ~130 GB/s;
    plain HWDGE/SWDGE dma_start ~130-180 GB/s.  So reads are trimmed by
    shrinking the PER-DESCRIPTOR row count: slot j reads 128*rpp[j] rows
    (rpp = ceil(group-max total/128) in {3,4} typically), baked in at
    compile time from the actual input (kernel() sorts the 64 batch rows
    by total and assigns rank 8j+i to core i slot j; recompiles if slot
    maxima change).  ~9.8 MB/core instead of 12.6.

  * PE segment-sum in bf16: out[t] = sum_g S[g,t]*enc[g] with the 0/1
    selection matrix built ON DEVICE in setup (like the baseline's
    coefficient tiles): S^T[t,g] = (g<cum[t]) - (g<cum[t]-len[t]) via
    DVE compares in token-major layout, PE-transposed to row-major lhsT.
    fp32r matmuls measured 3.5x slower than bf16 here (self-loading
    4-byte weights); f32 would be 4x.  enc is cast f32->bf16 on chip,
    split DVE/ACT (the SWDGE cast-DMA runs at half rate: 180 GB/s).
    Tokens interleave 2-per-partition (even/odd lhsT column split) so
    each store descriptor covers 2 tokens = 3 KB.

  * 1/len scaling + len==0 masking on eviction: ACT Copy with a
    per-partition scale column in {0,0.5,1}, PSUM -> SBUF bf16.

  * bf16 stores, 4 batch rows per store (HWDGE SP ring; loads go on the
    gpsimd SWDGE path).  HWDGE DMAs drain serially with ~2us fixed cost
    each, so fewer/bigger stores measured faster (42 us total vs 46 at
    2 rows/store).  Host upcasts to f32: rel-err ~2e-3 << the 2e-2
    gate, and write traffic halves.

In-loop budget per core: DMA ~9.8 MB read + ~3.1 MB write, PE ~46K
cycles (~19 us warm, measured 25 us solo), ACT ~18 us, DVE ~10 us.
Component medians: loads 29.7 us, loads+matmuls 31.5, +evicts 35.7,
full 42.3 us (vs 59.8 us baseline gather kernel).
"""

import numpy as np

import concourse.bacc as bacc
import concourse.bass as bass
import concourse.mybir as mybir
import concourse.tile as tile
from concourse.bass_utils import run_bass_kernel_spmd
from concourse.masks import make_identity

NCORES = 8
BZ, P, T, H = 64, 512, 256, 768
BL = BZ // NCORES  # batch rows per core
MAXR = 4  # max rows per load-descriptor (P / 128)
NCH = T // 128  # 128-token chunks per batch row

F32 = mybir.dt.float32
BF16 = mybir.dt.bfloat16
I32 = mybir.dt.int32
ALU = mybir.AluOpType
AF = mybir.ActivationFunctionType


def _sel_mask(nc, t, base):
    """t[k,p] = 1 iff base + k - 2p == 0."""
    nc.gpsimd.memset(t, 0.0)
    nc.gpsimd.affine_select(
        out=t, in_=t, compare_op=ALU.not_equal, fill=1.0,
        base=base, pattern=[[-2, 128]], channel_multiplier=1,
    )


def _tri_mask(nc, t):
    """t[k,p] = 1 iff k - p <= 0 (lower-left prefix mask for cumsum)."""
    nc.gpsimd.memset(t, 0.0)
    nc.gpsimd.affine_select(
        out=t, in_=t, compare_op=ALU.is_gt, fill=1.0,
        base=0, pattern=[[-1, 128]], channel_multiplier=1,
    )


def _slot_plan(lens_np):
    """Sort rows by total subwords; slot j takes ranks [8j, 8j+8).

    Loads must use full-128-offset indirect DMAs (partial-offset indirect
    and all plain DMA paths measured 2-4x below line rate), so the read
    trim happens via the rows-per-descriptor granularity: slot j reads
    128*rpp[j] rows with rpp[j] = ceil(group_max_total/128).

    Returns (plan, groups) with plan = (rpp, ktok): ktok[j] = store
    partitions (2 tokens each, equal within store pairs), groups[j][i] =
    original batch row handled by core i slot j.
    """
    totals = lens_np.sum(axis=1)
    tok_counts = (lens_np > 0).sum(axis=1)
    order = np.argsort(totals, kind="stable")
    rpp, ktok, groups = [], [], []
    for j in range(BL):
        grp = order[j * NCORES : (j + 1) * NCORES]
        rpp.append(int(max(1, np.ceil(totals[grp].max() / 128))))
        ktok.append(int(max(1, np.ceil(tok_counts[grp].max() / 2))))
        groups.append([int(g) for g in grp])
    # slots 2k, 2k+1 share one res tile / store -> common token count
    for k in range(BL // 2):
        m = max(ktok[2 * k], ktok[2 * k + 1])
        ktok[2 * k] = ktok[2 * k + 1] = m
    return (tuple(rpp), tuple(ktok)), groups


def _build_nc(plan, repeat=0, enc_bufs=3, res_bufs=3, asserts=True,
              parts="lmes", cast_dve_frac=0.5, store_eng="sync",
              store_rows=4):
    """parts: subset of l(oad) m(atmul) e(vict) s(tore) for timing."""
    rpp, ktok = plan
    nc = bacc.Bacc(
        "TRN2", target_bir_lowering=False, debug=False,
        num_devices=NCORES, enable_asserts=asserts,
    )
    enc = nc.dram_tensor("enc", [BL * P, H], F32, kind="ExternalInput").ap()
    lens = nc.dram_tensor("lens", [BL, T], I32, kind="ExternalInput").ap()
    out = nc.dram_tensor("out", [BL, T, H], BF16, kind="ExternalOutput").ap()

    with tile.TileContext(nc) as tc:
        with (
            tc.tile_pool(name="pers", bufs=1) as pers,
            tc.tile_pool(name="enc", bufs=enc_bufs) as apool,
            tc.tile_pool(name="res", bufs=res_bufs) as rpool,
        ):
            # persistent tiles: selection matrices, scales, load offsets
            S = [[pers.tile([128, T], BF16, tag=f"S{b}_{c}", name=f"S{b}_{c}")
                  for c in range(rpp[b])] for b in range(BL)]
            inv_e = pers.tile([128, BL], F32, tag="inv_e")
            inv_o = pers.tile([128, BL], F32, tag="inv_o")
            seqs = {}
            for r in sorted(set(rpp)):
                sq = pers.tile([128, BL], I32, tag=f"seq{r}", name=f"seq{r}")
                # seq_r[p, b] = b*P + r*p: row offsets, r rows/descriptor
                nc.gpsimd.iota(sq[:], pattern=[[P, BL]], base=0,
                               channel_multiplier=r)
                seqs[r] = sq

            # ---------------- setup (outside the timed loop) ----------------
            with (
                tc.tile_pool(name="scr", bufs=2) as spool,
                tc.tile_pool(name="scrc", bufs=1) as cpool,
                tc.tile_pool(name="spsum", bufs=1, space="PSUM") as sppool,
            ):
                ident = cpool.tile([128, 128], F32)
                make_identity(nc, ident[:])
                ones = cpool.tile([128, 128], F32)
                nc.gpsimd.memset(ones[:], 1.0)
                tri = cpool.tile([128, 128], F32)
                _tri_mask(nc, tri[:])
                se0 = cpool.tile([128, 128], F32)
                _sel_mask(nc, se0[:], base=0)
                se1 = cpool.tile([128, 128], F32)
                _sel_mask(nc, se1[:], base=128)
                so0 = cpool.tile([128, 128], F32)
                _sel_mask(nc, so0[:], base=-1)
                so1 = cpool.tile([128, 128], F32)
                _sel_mask(nc, so1[:], base=127)
                gr_i = cpool.tile([128, P], I32)
                nc.gpsimd.iota(gr_i[:], pattern=[[1, P]], base=0,
                               channel_multiplier=0)
                gr_f = cpool.tile([128, P], F32)
                nc.vector.tensor_copy(gr_f[:], gr_i[:])

                # lens -> lensT chunks [token, batch]
                lens_i = cpool.tile([BL, T], I32)
                nc.sync.dma_start(out=lens_i[:], in_=lens[:, :])
                lens_f = cpool.tile([BL, T], F32)
                nc.vector.tensor_copy(lens_f[:], lens_i[:])
                lensT = []
                for tcn in range(NCH):
                    ps_t = sppool.tile([128, BL], F32, tag="tr")
                    nc.tensor.transpose(
                        out=ps_t[:], in_=lens_f[:, tcn * 128 : (tcn + 1) * 128],
                        identity=ident[0:BL, 0:BL],
                    )
                    lt = cpool.tile([128, BL], F32, tag=f"lensT{tcn}")
                    nc.vector.tensor_copy(lt[:], ps_t[:])
                    lensT.append(lt)

                # cumT chunks: cum[t] = sum_{t'<=t} lens[t']
                cumT = []
                for tcn in range(NCH):
                    pc = sppool.tile([128, BL], F32, tag="cum")
                    if tcn == 0:
                        nc.tensor.matmul(out=pc[:], lhsT=tri[:], rhs=lensT[0][:],
                                         start=True, stop=True)
                    else:
                        nc.tensor.matmul(out=pc[:], lhsT=ones[:], rhs=lensT[0][:],
                                         start=True, stop=False)
                        nc.tensor.matmul(out=pc[:], lhsT=tri[:], rhs=lensT[1][:],
                                         start=False, stop=True)
                    ct = cpool.tile([128, BL], F32, tag=f"cumT{tcn}")
                    nc.vector.tensor_copy(ct[:], pc[:])
                    cumT.append(ct)
                clT = []
                for tcn in range(NCH):
                    cl = cpool.tile([128, BL], F32, tag=f"clT{tcn}")
                    nc.vector.tensor_sub(cl[:], cumT[tcn][:], lensT[tcn][:])
                    clT.append(cl)

                # per-parity token lens (pair layout) -> eviction scales
                def _pairlens(masks, name):
                    pt = sppool.tile([128, BL], F32, tag=name)
                    nc.tensor.matmul(out=pt[:], lhsT=masks[0][:], rhs=lensT[0][:],
                                     start=True, stop=False)
                    nc.tensor.matmul(out=pt[:], lhsT=masks[1][:], rhs=lensT[1][:],
                                     start=False, stop=True)
                    lv = cpool.tile([128, BL], F32, tag=f"{name}_s")
                    nc.vector.tensor_copy(lv[:], pt[:])
                    return lv

                l0v = _pairlens((se0, se1), "l0v")  # lens[2p]
                l1v = _pairlens((so0, so1), "l1v")  # lens[2p+1]

                def _inv(lv, dst):
                    # dst = (lv>0) - 0.5*(lv>1)  ->  {0:0, 1:1, 2:0.5}
                    a = spool.tile([128, BL], F32, tag="ab_a")
                    nc.vector.tensor_scalar(out=a[:], in0=lv[:], scalar1=1.0,
                                            scalar2=0.5, op0=ALU.is_gt,
                                            op1=ALU.mult)
                    g = spool.tile([128, BL], F32, tag="ab_g")
                    nc.vector.tensor_scalar(out=g[:], in0=lv[:], scalar1=0.0,
                                            scalar2=None, op0=ALU.is_gt)
                    nc.vector.tensor_sub(dst[:], g[:], a[:])

                _inv(l0v, inv_e)
                _inv(l1v, inv_o)

                # selection matrices: S^T[t,g] = (g<cum[t]) - (g<cum[t]-len[t])
                for b in range(BL):
                    w = rpp[b] * 128
                    for tcn in range(NCH):
                        aT = spool.tile([128, P], F32, tag="aT")
                        nc.vector.tensor_scalar(
                            out=aT[:, 0:w], in0=gr_f[:, 0:w],
                            scalar1=clT[tcn][:, b : b + 1], scalar2=None,
                            op0=ALU.is_lt)
                        sT = spool.tile([128, P], F32, tag="sT")
                        nc.vector.scalar_tensor_tensor(
                            out=sT[:, 0:w], in0=gr_f[:, 0:w],
                            scalar=cumT[tcn][:, b : b + 1], in1=aT[:, 0:w],
                            op0=ALU.is_lt, op1=ALU.subtract)
                        sTr = sT[:, 0:w].rearrange("t (g c) -> t c g",
                                                   c=rpp[b])
                        for c in range(rpp[b]):
                            pt = sppool.tile([128, 128], F32, tag="tp")
                            nc.tensor.transpose(
                                out=pt[:], in_=sTr[:, c, :],
                                identity=ident[:])
                            nc.vector.tensor_copy(
                                S[b][c][:, tcn * 128 : (tcn + 1) * 128], pt[:])

            # ---------------- timed steady-state loop ----------------
            with tc.tile_pool(name="mm", bufs=2, space="PSUM") as mpool:

                def main_body(_iv=None):
                    res = None
                    for b in range(BL):
                        r, kt = rpp[b], ktok[b]
                        w = r * H
                        a = apool.tile([128, MAXR * H], BF16, tag="enc")
                        if "l" in parts:
                            af = apool.tile([128, MAXR * H], F32, tag="encf")
                            nc.gpsimd.indirect_dma_start(
                                out=af[:, 0:w], out_offset=None,
                                in_=enc[:, :],
                                in_offset=bass.IndirectOffsetOnAxis(
                                    ap=seqs[r][:, b : b + 1], axis=0))
                            split = int(w * cast_dve_frac) // H * H
                            if split > 0:
                                nc.vector.tensor_copy(a[:, 0:split],
                                                      af[:, 0:split])
                            if split < w:
                                nc.scalar.activation(
                                    out=a[:, split:w], in_=af[:, split:w],
                                    func=AF.Copy)
                        elif "m" in parts:
                            # bench-only: satisfy write-before-read
                            nc.gpsimd.memset(a[:, 0:w], 0.25)
                        if b % store_rows == 0:
                            res = rpool.tile([128, store_rows * 2 * H], BF16,
                                             tag="res")
                        rhalf = (b % store_rows) * 2 * H
                        for par, inv in ((0, inv_e), (1, inv_o)):
                            pa = mpool.tile([128, 512], F32, tag=f"p{par}a")
                            pb = mpool.tile([128, H - 512], F32, tag=f"p{par}b")
                            if "m" in parts:
                                for c in range(r):
                                    lhsT = (
                                        S[b][c][:, :]
                                        .rearrange("g (t q) -> g q t", q=2)
                                        [:, par, :])
                                    rhs = a[:, c * H : (c + 1) * H]
                                    nc.tensor.matmul(
                                        out=pa[:], lhsT=lhsT, rhs=rhs[:, 0:512],
                                        start=(c == 0), stop=(c == r - 1))
                                    nc.tensor.matmul(
                                        out=pb[:], lhsT=lhsT, rhs=rhs[:, 512:H],
                                        start=(c == 0), stop=(c == r - 1))
                            if "e" in parts:
                                col = inv[:, b : b + 1]
                                base = rhalf + par * H
                                nc.scalar.activation(
                                    out=res[:, base : base + 512],
                                    in_=pa[:], func=AF.Copy, scale=col)
                                nc.scalar.activation(
                                    out=res[:, base + 512 : base + H],
                                    in_=pb[:], func=AF.Copy, scale=col)
                        if "s" in parts and b % store_rows == store_rows - 1:
                            b0 = b - store_rows + 1
                            kts = max(ktok[j] for j in range(b0, b + 1))
                            dest = out[b0 : b + 1, 0 : 2 * kts, :].rearrange(
                                "o (p q) h -> p o q h", q=2)
                            seng = (nc.sync if store_eng == "sync" else
                                    nc.scalar if store_eng == "scalar" else
                                    (nc.sync if (b // store_rows) % 2 == 0
                                     else nc.scalar))
                            seng.dma_start(
                                out=dest,
                                in_=res[0:kts, :].rearrange(
                                    "p (o q h) -> p o q h", q=2, h=H))

                if repeat:
                    tc.For_i_unrolled(0, repeat, 1, main_body, max_unroll=4)
                else:
                    main_body()

    nc.compile()
    return nc


_NC_CACHE = {}


def _get_nc(plan):
    if plan not in _NC_CACHE:
        _NC_CACHE[plan] = _build_nc(plan)
    return _NC_CACHE[plan]


def _make_in_maps(enc_np, lens_np, groups):
    in_maps = []
    for i in range(NCORES):
        rows = [groups[j][i] for j in range(BL)]
        in_maps.append({
            "enc": np.ascontiguousarray(
                enc_np[rows].reshape(BL * P, H)),
            "lens": np.ascontiguousarray(lens_np[rows]),
        })
    return in_maps


def kernel(enc_out, bert_mask, bert_lens):
    del bert_mask  # implied by bert_lens
    enc_np = np.ascontiguousarray(np.asarray(enc_out, dtype=np.float32))
    lens_np = np.ascontiguousarray(np.asarray(bert_lens, dtype=np.int32))
    assert enc_np.shape == (BZ, P, H) and lens_np.shape == (BZ, T)

    plan, groups = _slot_plan(lens_np)
    nc = _get_nc(plan)
    in_maps = _make_in_maps(enc_np, lens_np, groups)
    results = run_bass_kernel_spmd(nc, in_maps, core_ids=list(range(NCORES))).results

    ktok = plan[1]
    final = np.zeros((BZ, T, H), dtype=np.float32)
    for i in range(NCORES):
        dev = np.asarray(results[i]["out"])
        for j in range(BL):
            row = groups[j][i]
            nt = 2 * ktok[j]
            final[row, 0:nt] = dev[j, 0:nt].astype(np.float32)
    return final


# revision 8
# speedup vs baseline: 2.1811x; 1.0339x over previous
"""Trainium2 Bass kernel for nn_BertEmbedding_1623497638029 (PE version).

Per batch row b and token t (T=256 tokens, P=512 subword positions,
H=768), with subword counts lens in {0,1,2}:

    out[b,t] = mean(enc[b, cum[t]-lens[t] : cum[t]]) if lens[t] else 0

Strategy (pure data parallel, 8 rows/core, no cross-core comms):

  * Loads: full-128-offset indirect DMA (iota offsets, sequential rows).
    Measured DMA rates on this hardware for the same bytes/descriptors:
    indirect+128 offsets 353 GB/s; indirect with <128 offsets ~130 GB/s;
    plain HWDGE/SWDGE dma_start ~130-180 GB/s.  So reads are trimmed by
    shrinking the PER-DESCRIPTOR row count: slot j reads 128*rpp[j] rows
    (rpp = ceil(group-max total/128) in {3,4} typically), baked in at
    compile time from the actual input (kernel() sorts the 64 batch rows
    by total and assigns rank 8j+i to core i slot j; recompiles if slot
    maxima change).  ~9.8 MB/core instead of 12.6.

  * PE segment-sum in bf16: out[t] = sum_g S[g,t]*enc[g] with the 0/1
    selection matrix built ON DEVICE in setup (like the baseline's
    coefficient tiles): S^T[t,g] = (g<cum[t]) - (g<cum[t]-len[t]) via
    DVE compares in token-major layout, PE-transposed to row-major lhsT.
    fp32r matmuls measured 3.5x slower than bf16 here (self-loading
    4-byte weights); f32 would be 4x.  enc is cast f32->bf16 on chip,
    split DVE/ACT (the SWDGE cast-DMA runs at half rate: 180 GB/s).
    Tokens interleave 2-per-partition (even/odd lhsT column split) so
    each store descriptor covers 2 tokens = 3 KB.

  * 1/len scaling + len==0 masking on eviction: ACT Copy with a
    per-partition scale column in {0,0.5,1}, PSUM -> SBUF bf16.

  * bf16 stores, 4 batch rows per store (HWDGE SP ring; loads go on the
    gpsimd SWDGE path).  HWDGE DMAs drain serially with ~2us fixed cost
    each, so fewer/bigger stores measured faster (42 us total vs 46 at
    2 rows/store).  Host upcasts to f32: rel-err ~2e-3 << the 2e-2
    gate, and write traffic halves.

In-loop budget per core: DMA ~9.8 MB read + ~3.1 MB write, PE ~46K
cycles (~19 us warm, measured 25 us solo), ACT ~18 us, DVE ~10 us.
Component medians: loads 29.7 us, loads+matmuls 31.5, +evicts 35.7,
full 42.3 us (vs 59.8 us baseline gather kernel).
"""

import numpy as np

import concourse.bacc as bacc
import concourse.bass as bass
import concourse.mybir as mybir
import concourse.tile as tile
from concourse.bass_utils import run_bass_kernel_spmd
from concourse.masks import make_identity

NCORES = 8
BZ, P, T, H = 64, 512, 256, 768
BL = BZ // NCORES  # batch rows per core
MAXR = 4  # max rows per load-descriptor (P / 128)
NCH = T // 128  # 128-token chunks per batch row

F32 = mybir.dt.float32
BF16 = mybir.dt.bfloat16
I32 = mybir.dt.int32
ALU = mybir.AluOpType
AF = mybir.ActivationFunctionType


def _sel_mask(nc, t, base):
    """t[k,p] = 1 iff base + k - 2p == 0."""
    nc.gpsimd.memset(t, 0.0)
    nc.gpsimd.affine_select(
        out=t, in_=t, compare_op=ALU.not_equal, fill=1.0,
        base=base, pattern=[[-2, 128]], channel_multiplier=1,
    )


def _tri_mask(nc, t):
    """t[k,p] = 1 iff k - p <= 0 (lower-left prefix mask for cumsum)."""
    nc.gpsimd.memset(t, 0.0)
    nc.gpsimd.affine_select(
        out=t, in_=t, compare_op=ALU.is_gt, fill=1.0,
        base=0, pattern=[[-1, 128]], channel_multiplier=1,
    )


def _slot_plan(lens_np):
    """Sort rows by total subwords; slot j takes ranks [8j, 8j+8).

    Loads must use full-128-offset indirect DMAs (partial-offset indirect
    and all plain DMA paths measured 2-4x below line rate), so the read
    trim happens via the rows-per-descriptor granularity: slot j reads
    128*rpp[j] rows with rpp[j] = ceil(group_max_total/128).

    Returns (plan, groups) with plan = (rpp, ktok): ktok[j] = store
    partitions (2 tokens each, equal within store pairs), groups[j][i] =
    original batch row handled by core i slot j.
    """
    totals = lens_np.sum(axis=1)
    tok_counts = (lens_np > 0).sum(axis=1)
    order = np.argsort(totals, kind="stable")
    rpp, ktok, groups = [], [], []
    for j in range(BL):
        grp = order[j * NCORES : (j + 1) * NCORES]
        rpp.append(int(max(1, np.ceil(totals[grp].max() / 128))))
        ktok.append(int(max(1, np.ceil(tok_counts[grp].max() / 2))))
        groups.append([int(g) for g in grp])
    # slots 2k, 2k+1 share one res tile / store -> common token count
    for k in range(BL // 2):
        m = max(ktok[2 * k], ktok[2 * k + 1])
        ktok[2 * k] = ktok[2 * k + 1] = m
    return (tuple(rpp), tuple(ktok)), groups


def _build_nc(plan, repeat=0, enc_bufs=3, res_bufs=3, asserts=True,
              parts="lmes", cast_dve_frac=0.5, store_eng="sync",
              store_rows=4):
    """parts: subset of l(oad) m(atmul) e(vict) s(tore) for timing."""
    rpp, ktok = plan
    nc = bacc.Bacc(
        "TRN2", target_bir_lowering=False, debug=False,
        num_devices=NCORES, enable_asserts=asserts,
    )
    enc = nc.dram_tensor("enc", [BL * P, H], F32, kind="ExternalInput").ap()
    lens = nc.dram_tensor("lens", [BL, T], I32, kind="ExternalInput").ap()
    out = nc.dram_tensor("out", [BL, T, H], BF16, kind="ExternalOutput").ap()

    with tile.TileContext(nc) as tc:
        with (
            tc.tile_pool(name="pers", bufs=1) as pers,
            tc.tile_pool(name="enc", bufs=enc_bufs) as apool,
            tc.tile_pool(name="res", bufs=res_bufs) as rpool,
        ):
            # persistent tiles: selection matrices, scales, load offsets
            S = [[pers.tile([128, T], BF16, tag=f"S{b}_{c}", name=f"S{b}_{c}")
                  for c in range(rpp[b])] for b in range(BL)]
            inv_e = pers.tile([128, BL], F32, tag="inv_e")
            inv_o = pers.tile([128, BL], F32, tag="inv_o")
            seqs = {}
            for r in sorted(set(rpp)):
                sq = pers.tile([128, BL], I32, tag=f"seq{r}", name=f"seq{r}")
                # seq_r[p, b] = b*P + r*p: row offsets, r rows/descriptor
                nc.gpsimd.iota(sq[:], pattern=[[P, BL]], base=0,
                               channel_multiplier=r)
                seqs[r] = sq

            # ---------------- setup (outside the timed loop) ----------------
            with (
                tc.tile_pool(name="scr", bufs=2) as spool,
                tc.tile_pool(name="scrc", bufs=1) as cpool,
                tc.tile_pool(name="spsum", bufs=1, space="PSUM") as sppool,
            ):
                ident = cpool.tile([128, 128], F32)
                make_identity(nc, ident[:])
                ones = cpool.tile([128, 128], F32)
                nc.gpsimd.memset(ones[:], 1.0)
                tri = cpool.tile([128, 128], F32)
                _tri_mask(nc, tri[:])
                se0 = cpool.tile([128, 128], F32)
                _sel_mask(nc, se0[:], base=0)
                se1 = cpool.tile([128, 128], F32)
                _sel_mask(nc, se1[:], base=128)
                so0 = cpool.tile([128, 128], F32)
                _sel_mask(nc, so0[:], base=-1)
                so1 = cpool.tile([128, 128], F32)
                _sel_mask(nc, so1[:], base=127)
                gr_i = cpool.tile([128, P], I32)
                nc.gpsimd.iota(gr_i[:], pattern=[[1, P]], base=0,
                               channel_multiplier=0)
                gr_f = cpool.tile([128, P], F32)
                nc.vector.tensor_copy(gr_f[:], gr_i[:])

                # lens -> lensT chunks [token, batch]
                lens_i = cpool.tile([BL, T], I32)
                nc.sync.dma_start(out=lens_i[:], in_=lens[:, :])
                lens_f = cpool.tile([BL, T], F32)
                nc.vector.tensor_copy(lens_f[:], lens_i[:])
                lensT = []
                for tcn in range(NCH):
                    ps_t = sppool.tile([128, BL], F32, tag="tr")
                    nc.tensor.transpose(
                        out=ps_t[:], in_=lens_f[:, tcn * 128 : (tcn + 1) * 128],
                        identity=ident[0:BL, 0:BL],
                    )
                    lt = cpool.tile([128, BL], F32, tag=f"lensT{tcn}")
                    nc.vector.tensor_copy(lt[:], ps_t[:])
                    lensT.append(lt)

                # cumT chunks: cum[t] = sum_{t'<=t} lens[t']
                cumT = []
                for tcn in range(NCH):
                    pc = sppool.tile([128, BL], F32, tag="cum")
                    if tcn == 0:
                        nc.tensor.matmul(out=pc[:], lhsT=tri[:], rhs=lensT[0][:],
                                         start=True, stop=True)
                    else:
                        nc.tensor.matmul(out=pc[:], lhsT=ones[:], rhs=lensT[0][:],
                                         start=True, stop=False)
                        nc.tensor.matmul(out=pc[:], lhsT=tri[:], rhs=lensT[1][:],
                                         start=False, stop=True)
                    ct = cpool.tile([128, BL], F32, tag=f"cumT{tcn}")
                    nc.vector.tensor_copy(ct[:], pc[:])
                    cumT.append(ct)
                clT = []
                for tcn in range(NCH):
                    cl = cpool.tile([128, BL], F32, tag=f"clT{tcn}")
                    nc.vector.tensor_sub(cl[:], cumT[tcn][:], lensT[tcn][:])
                    clT.append(cl)

                # per-parity token lens (pair layout) -> eviction scales
                def _pairlens(masks, name):
                    pt = sppool.tile([128, BL], F32, tag=name)
                    nc.tensor.matmul(out=pt[:], lhsT=masks[0][:], rhs=lensT[0][:],
                                     start=True, stop=False)
                    nc.tensor.matmul(out=pt[:], lhsT=masks[1][:], rhs=lensT[1][:],
                                     start=False, stop=True)
                    lv = cpool.tile([128, BL], F32, tag=f"{name}_s")
                    nc.vector.tensor_copy(lv[:], pt[:])
                    return lv

                l0v = _pairlens((se0, se1), "l0v")  # lens[2p]
                l1v = _pairlens((so0, so1), "l1v")  # lens[2p+1]

                def _inv(lv, dst):
                    # dst = (lv>0) - 0.5*(lv>1)  ->  {0:0, 1:1, 2:0.5}
                    a = spool.tile([128, BL], F32, tag="ab_a")
                    nc.vector.tensor_scalar(out=a[:], in0=lv[:], scalar1=1.0,
                                            scalar2=0.5, op0=ALU.is_gt,
                                            op1=ALU.mult)
                    g = spool.tile([128, BL], F32, tag="ab_g")
                    nc.vector.tensor_scalar(out=g[:], in0=lv[:], scalar1=0.0,
                                            scalar2=None, op0=ALU.is_gt)
                    nc.vector.tensor_sub(dst[:], g[:], a[:])

                _inv(l0v, inv_e)
                _inv(l1v, inv_o)

                # selection matrices: S^T[t,g] = (g<cum[t]) - (g<cum[t]-len[t])
                for b in range(BL):
                    w = rpp[b] * 128
                    for tcn in range(NCH):
                        aT = spool.tile([128, P], F32, tag="aT")
                        nc.vector.tensor_scalar(
                            out=aT[:, 0:w], in0=gr_f[:, 0:w],
                            scalar1=clT[tcn][:, b : b + 1], scalar2=None,
                            op0=ALU.is_lt)
                        sT = spool.tile([128, P], F32, tag="sT")
                        nc.vector.scalar_tensor_tensor(
                            out=sT[:, 0:w], in0=gr_f[:, 0:w],
                            scalar=cumT[tcn][:, b : b + 1], in1=aT[:, 0:w],
                            op0=ALU.is_lt, op1=ALU.subtract)
                        sTr = sT[:, 0:w].rearrange("t (g c) -> t c g",
                                                   c=rpp[b])
                        for c in range(rpp[b]):
                            pt = sppool.tile([128, 128], F32, tag="tp")
                            nc.tensor.transpose(
                                out=pt[:], in_=sTr[:, c, :],
                                identity=ident[:])
                            nc.vector.tensor_copy(
                                S[b][c][:, tcn * 128 : (tcn + 1) * 128], pt[:])

            # ---------------- timed steady-state loop ----------------
            with tc.tile_pool(name="mm", bufs=2, space="PSUM") as mpool:

                def main_body(_iv=None):
                    res = None
                    for b in range(BL):
                        r, kt = rpp[b], ktok[b]
                        w = r * H
                        a = apool.tile([128, MAXR * H], BF16, tag="enc")
                        if "l" in parts:
                            af = apool.tile([128, MAXR * H], F32, tag="encf")
                            nc.gpsimd.indirect_dma_start(
                                out=af[:, 0:w], out_offset=None,
                                in_=enc[:, :],
                                in_offset=bass.IndirectOffsetOnAxis(
                                    ap=seqs[r][:, b : b + 1], axis=0))
                            split = int(w * cast_dve_frac) // H * H
                            if split > 0:
                                nc.vector.tensor_copy(a[:, 0:split],
                                                      af[:, 0:split])
                            if split < w:
                                nc.scalar.activation(
                                    out=a[:, split:w], in_=af[:, split:w],
                                    func=AF.Copy)
                        elif "m" in parts:
                            # bench-only: satisfy write-before-read
                            nc.gpsimd.memset(a[:, 0:w], 0.25)
                        if b % store_rows == 0:
                            res = rpool.tile([128, store_rows * 2 * H], BF16,
                                             tag="res")
                        rhalf = (b % store_rows) * 2 * H
                        for par, inv in ((0, inv_e), (1, inv_o)):
                            pa = mpool.tile([128, 512], F32, tag=f"p{par}a")
                            pb = mpool.tile([128, H - 512], F32, tag=f"p{par}b")
                            if "m" in parts:
                                for c in range(r):
                                    lhsT = (
                                        S[b][c][:, :]
                                        .rearrange("g (t q) -> g q t", q=2)
                                        [:, par, :])
                                    rhs = a[:, c * H : (c + 1) * H]
                                    nc.tensor.matmul(
                                        out=pa[:], lhsT=lhsT, rhs=rhs[:, 0:512],
                                        start=(c == 0), stop=(c == r - 1))
                                    nc.tensor.matmul(
                                        out=pb[:], lhsT=lhsT, rhs=rhs[:, 512:H],
                                        start=(c == 0), stop=(c == r - 1))
                            if "e" in parts:
                                col = inv[:, b : b + 1]
                                base = rhalf + par * H
                                nc.scalar.activation(
                                    out=res[:, base : base + 512],
                                    in_=pa[:], func=AF.Copy, scale=col)
                                # small half on DVE: balances ACT (cast+evict)
                                nc.vector.tensor_scalar(
                                    out=res[:, base + 512 : base + H],
                                    in0=pb[:], scalar1=col, scalar2=None,
                                    op0=ALU.mult)
                        if "s" in parts and b % store_rows == store_rows - 1:
                            b0 = b - store_rows + 1
                            kts = max(ktok[j] for j in range(b0, b + 1))
                            dest = out[b0 : b + 1, 0 : 2 * kts, :].rearrange(
                                "o (p q) h -> p o q h", q=2)
                            seng = (nc.sync if store_eng == "sync" else
                                    nc.scalar if store_eng == "scalar" else
                                    (nc.sync if (b // store_rows) % 2 == 0
                                     else nc.scalar))
                            seng.dma_start(
                                out=dest,
                                in_=res[0:kts, :].rearrange(
                                    "p (o q h) -> p o q h", q=2, h=H))

                if repeat:
                    tc.For_i_unrolled(0, repeat, 1, main_body, max_unroll=4)
                else:
                    main_body()

    nc.compile()
    return nc


_NC_CACHE = {}


def _get_nc(plan):
    if plan not in _NC_CACHE:
        _NC_CACHE[plan] = _build_nc(plan)
    return _NC_CACHE[plan]


def _make_in_maps(enc_np, lens_np, groups):
    in_maps = []
    for i in range(NCORES):
        rows = [groups[j][i] for j in range(BL)]
        in_maps.append({
            "enc": np.ascontiguousarray(
                enc_np[rows].reshape(BL * P, H)),
            "lens": np.ascontiguousarray(lens_np[rows]),
        })
    return in_maps


def kernel(enc_out, bert_mask, bert_lens):
    del bert_mask  # implied by bert_lens
    enc_np = np.ascontiguousarray(np.asarray(enc_out, dtype=np.float32))
    lens_np = np.ascontiguousarray(np.asarray(bert_lens, dtype=np.int32))
    assert enc_np.shape == (BZ, P, H) and lens_np.shape == (BZ, T)

    plan, groups = _slot_plan(lens_np)
    nc = _get_nc(plan)
    in_maps = _make_in_maps(enc_np, lens_np, groups)
    results = run_bass_kernel_spmd(nc, in_maps, core_ids=list(range(NCORES))).results

    ktok = plan[1]
    final = np.zeros((BZ, T, H), dtype=np.float32)
    for i in range(NCORES):
        dev = np.asarray(results[i]["out"])
        for j in range(BL):
            row = groups[j][i]
            nt = 2 * ktok[j]
            final[row, 0:nt] = dev[j, 0:nt].astype(np.float32)
    return final
